# revision 38
# baseline (speedup 1.0000x reference)
"""Sliding-window causal attention (B=2,T=2048,C=1024,H=16,D=64,W=256) on 8 trn2 cores.

Sharding: core c = (batch b = c//4, head-group g = c%4 of 4 heads).
Each core computes q/k/v projections for its 4 heads on its batch, windowed
attention per head, and a partial output projection (its 256 channels of the
contraction); host sums the 4 partials per batch.

Fully interleaved single-stream schedule (v2):
  - xT shipped chunk-major (4 chunks of 512 token-columns x all 8 c-blocks)
    so the first projection group unblocks after ~1.5MB of DMA, not 4MB.
  - warmup matmuls on the mask tile keep the PE HAM clock-gate at 8/8.
  - per chunk: qk projections (both pairs), v tiles, then attention steps
    whose deps are satisfied; attention steps carry scores, PV, softmax
    normalization, out-projection and y DMA in a software pipeline so the
    PE queue never starves and output streams out continuously.
  - PSUM: "sc" ring 2x[128,1024] (4 banks: scores/proj/outproj/warmup) +
    "pv" ring 4x[65,512] (4 banks: PV accumulators w/ ones-row denominator).
  - softmax: scores^T [k,q]; denominators via ones-column folded into PV;
    per block: batched d [4,512] -> fast reciprocal -> DRAM-bounce
    partition-broadcast -> normalize into aT; outproj one step later covers
    the chain latency.
"""

import os
import sys

sys.path.insert(0, "/opt/trn_rl_repo")

import numpy as np
import ml_dtypes

import concourse.bass as bass
import concourse.tile as tile
from concourse import bacc
from concourse import mybir
from concourse.bass import ds, ts

BF16 = ml_dtypes.bfloat16

B, T, C = 2, 2048, 1024
H, W, D = 16, 256, 64
HPC = 4          # heads per core
CL = HPC * D     # 256 local channels per core
NKT = C // 128   # 8 contraction tiles for projections
NT = T // 128    # 16 token tiles
NCH = 4          # token-column chunks (512 cols each)
SCALE = 0.125    # 1/sqrt(D)
F32 = mybir.dt.float32
BF = mybir.dt.bfloat16
N_WARM = int(os.environ.get("ATTN_WARMUP_MMS", "10"))
# NOTE: gpsimd partition_broadcast gives WRONG RESULTS on hardware (while
# correct in CoreSim) — the DRAM-bounce broadcast is used instead.


def build_program():
    nc = bacc.Bacc("TRN2", target_bir_lowering=False, debug=False)

    xT_d = nc.dram_tensor("xTt", [128, NKT * T], BF, kind="ExternalInput")
    w_d = nc.dram_tensor("wt", [128, 3 * NKT * CL], BF, kind="ExternalInput")
    wo_d = nc.dram_tensor("wot", [128, 2 * C], BF, kind="ExternalInput")
    mask_d = nc.dram_tensor("maskt", [128, 512], BF, kind="ExternalInput")
    y_d = nc.dram_tensor("y", [T, C], BF, kind="ExternalOutput")

    with tile.TileContext(nc) as tc:
        with (
            tc.tile_pool(name="const", bufs=1) as constp,
            tc.tile_pool(name="acts", bufs=1) as actsp,
            tc.tile_pool(name="epool", bufs=20) as ep,
            tc.tile_pool(name="ysb", bufs=3) as yp,
            tc.tile_pool(name="scps", bufs=2, space="PSUM") as scp,
            tc.tile_pool(name="pvps", bufs=4, space="PSUM") as pvp,
        ):
            # ---- static SBUF tiles + priority-ordered loads ----
            mask_all = constp.tile([128, 512], BF, tag="maskall", name="mask_all")
            w_all = constp.tile([128, 3 * NKT * CL], BF, tag="wall", name="w_all")
            wo_all = constp.tile([128, 2 * C], BF, tag="woall", name="wo_all")
            xT_all = constp.tile([128, NKT * T], BF, tag="xTall", name="xT_all")
            WQC = NKT * CL  # 2048 cols per projection weight

            # two parallel HW DMA queues (sync + scalar): weights mostly on
            # the scalar queue, activation chunks on sync, ordered by need
            nc.sync.dma_start(w_all[:, ds(0, WQC)], w_d[:, ds(0, WQC)])  # wq
            nc.scalar.dma_start(mask_all[:], mask_d[:])
            nc.scalar.dma_start(w_all[:, ds(WQC, WQC)], w_d[:, ds(WQC, WQC)])  # wk
            # chunk0 split in two so the very first matmuls start earlier
            nc.sync.dma_start(xT_all[:, ds(0, 2048)], xT_d[:, ds(0, 2048)])
            nc.sync.dma_start(xT_all[:, ds(2048, 2048)], xT_d[:, ds(2048, 2048)])
            nc.scalar.dma_start(w_all[:, ds(2 * WQC, WQC)], w_d[:, ds(2 * WQC, WQC)])
            nc.sync.dma_start(xT_all[:, ds(4096, 4096)], xT_d[:, ds(4096, 4096)])
            nc.scalar.dma_start(xT_all[:, ds(8192, 4096)], xT_d[:, ds(8192, 4096)])
            nc.scalar.dma_start(wo_all[:], wo_d[:])
            nc.sync.dma_start(xT_all[:, ds(12288, 4096)], xT_d[:, ds(12288, 4096)])

            wq_sb = [w_all[:, ds((0 * NKT + i) * CL, CL)] for i in range(NKT)]
            wk_sb = [w_all[:, ds((1 * NKT + i) * CL, CL)] for i in range(NKT)]
            wv_sb = [w_all[:, ds((2 * NKT + i) * CL, CL)] for i in range(NKT)]
            wo_sb = [wo_all[:, ds(j * C, C)] for j in range(2)]

            def xchunk(n, kt, off=0, width=512):
                """x^T block: contraction tile kt, token cols [512n+off, +width)."""
                return xT_all[:, ds((n * NKT + kt) * 512 + off, width)]

            # host layout [maskd|masks|maskd|masks]
            maskf_v = mask_all.rearrange("p (b s x) -> p b s x", b=2, s=2)
            maskd_v = maskf_v[:, :, 0, :]

            # persistent activations
            qT_sb = [actsp.tile([128, T], BF, tag=f"qT{m}", name=f"qT{m}") for m in range(2)]
            kT_sb = [actsp.tile([128, T], BF, tag=f"kT{m}", name=f"kT{m}") for m in range(2)]
            # aT split per 512-col block so outproj(b) has no false dep on
            # norm(b') for b' > b (cover scheduling relies on independence)
            aT_sb = [
                [
                    actsp.tile([128, 512], BF, tag=f"aT{m}b{b}", name=f"aT{m}b{b}")
                    for b in range(4)
                ]
                for m in range(2)
            ]
            # v natural layout, per token-tile: [v_h0(64)|1|v_h1(64)|1|...] = 260 cols
            v_sb = [actsp.tile([128, 4 * 65], BF, tag=f"v{t}", name=f"v{t}") for t in range(NT)]
            for t in range(NT):
                vv = v_sb[t].rearrange("p (h c) -> p h c", h=4)
                nc.gpsimd.memset(vv[:, :, 64:65], 1.0)
            # softmax denominators, head h at (row, coloff): matmul operands
            # must base at partition 0/32/64, so h3 lives at (64, 512:1024)
            DROW = ((0, 0), (32, 0), (64, 0), (64, 512))
            dnm = actsp.tile([128, 1024], F32, tag="dnm", name="dnm")
            rcp = actsp.tile([128, 1024], F32, tag="rcp", name="rcp")
            nc.gpsimd.memset(dnm[:], 1.0)
            ones_sb = actsp.tile([128, 64], F32, tag="ones", name="ones_sb")
            nc.gpsimd.memset(ones_sb[:], 1.0)

            # ---- PE warmup during input load (HAM at 2.4GHz) ----
            # runs on a memset scratch tile: no DMA dependency, so the PE
            # warms during the engine preamble + input load
            if N_WARM:
                warm_sb = actsp.tile([128, 512], BF, tag="warm", name="warm_sb")
                nc.gpsimd.memset(warm_sb[:], 0.5)
                wps = scp.tile([128, 1024], F32, tag="sc", name="ps_warm")
                for _ in range(N_WARM):
                    nc.tensor.matmul(
                        wps[:, 0:512],
                        lhsT=warm_sb[:, 0:128],
                        rhs=warm_sb[:],
                        start=True,
                        stop=True,
                    )

            # ---- projection helpers ----
            def qk_group(m, proj, n):
                """qT or kT, head-pair m, token-column chunk n (512 cols)."""
                w_sb, dstT = ((wq_sb, qT_sb), (wk_sb, kT_sb))[proj]
                ps = scp.tile([128, 1024], F32, tag="sc", name="ps_proj")
                for kt in range(NKT):
                    nc.tensor.matmul(
                        ps[:, 0:512],
                        lhsT=w_sb[kt][:, ts(m, 128)],
                        rhs=xchunk(n, kt),
                        start=(kt == 0),
                        stop=(kt == NKT - 1),
                    )
                nc.scalar.copy(dstT[m][:, ts(n, 512)], ps[:, 0:512])

            def v_tile(t):
                n, i = divmod(t, 4)
                ps = scp.tile([128, 1024], F32, tag="sc", name="ps_v")
                for kt in range(NKT):
                    nc.tensor.matmul(
                        ps[:, 0:CL],
                        lhsT=xchunk(n, kt, off=i * 128, width=128),
                        rhs=wv_sb[kt][:],
                        start=(kt == 0),
                        stop=(kt == NKT - 1),
                    )
                vv = v_sb[t].rearrange("p (h c) -> p h c", h=4)
                nc.vector.tensor_copy(
                    vv[:, :, 0:64], ps[:, 0:CL].rearrange("p (h c) -> p h c", h=4)[:]
                )

            # ---- attention state ----
            sc_ps = {}     # (mp, kt) -> scores psum tile
            e_tiles = {}   # (mp, kt) -> E sbuf tile
            pv_ps = {}     # h -> current PV psum tile

            def scores_mm(kt):
                nkt = 128 * min(3, NT - kt)
                for mp in range(2):
                    sc = scp.tile([128, 1024], F32, tag="sc", name="ps_sc")
                    for half in range(2):
                        rows = slice(64 * half, 64 * half + 64)
                        nc.tensor.matmul(
                            sc[:, ds(512 * half, nkt)],
                            lhsT=kT_sb[mp][rows, ts(kt, 128)],
                            rhs=qT_sb[mp][rows, ds(128 * kt, nkt)],
                            start=True,
                            stop=True,
                        )
                    sc_ps[(mp, kt)] = sc

            def exp_mask(kt):
                nkt = 128 * min(3, NT - kt)
                for mp in range(2):
                    sc = sc_ps.pop((mp, kt))
                    E = ep.tile([128, 768], BF, tag="E", name="E")
                    scv = sc.rearrange("p (b x) -> p b x", b=2)
                    Ev = E.rearrange("p (b x) -> p b x", b=2)
                    nc.scalar.activation(
                        Ev[:, :, 0:nkt],
                        scv[:, :, 0:nkt],
                        mybir.ActivationFunctionType.Exp,
                        scale=SCALE,
                    )
                    # fused mask: diag (d=0) + strict (d=2) of both heads;
                    # pair 1 masks on gpsimd to offload the vector engine
                    masker = nc.vector if mp == 0 else nc.gpsimd
                    if kt <= NT - 3:
                        Em = E.rearrange("p (b s x) -> p b s x", b=2, s=3)[
                            :, :, ::2, :
                        ]
                        masker.tensor_mul(Em[:], Em[:], maskf_v[:])
                    else:
                        masker.tensor_mul(
                            Ev[:, :, 0:128], Ev[:, :, 0:128], maskd_v[:]
                        )
                    e_tiles[(mp, kt)] = E

            def pv_step(j):
                """PV accumulation for query tile j, all 4 heads."""
                col = 128 * (j % 4)
                kts = [k2 for k2 in (j - 2, j - 1, j) if k2 >= 0]
                for h in range(4):
                    mp, hp = h // 2, h % 2
                    if j % 4 == 0:
                        pv_ps[h] = pvp.tile([65, 512], F32, tag="pv", name="ps_pv")
                    psh = pv_ps[h]
                    for idx, k2 in enumerate(kts):
                        nc.tensor.matmul(
                            psh[:, ds(col, 128)],
                            lhsT=v_sb[k2][:, ds(65 * h, 65)],
                            rhs=e_tiles[(mp, k2)][:, ds(384 * hp + 128 * (j - k2), 128)],
                            start=(idx == 0),
                            stop=(idx == len(kts) - 1),
                        )

            def norm_start(b, c0=0, c1=512):
                """Stage 1: denominator rows out, UNNORMALIZED evac of the PV
                psum into aT (frees the PV bank fast), batched reciprocal."""
                w = c1 - c0
                cs = ds(c0, w)
                for h in range(4):
                    mp, hp = h // 2, h % 2
                    row, co = DROW[h]
                    dd = dnm[row : row + 1, ds(co + c0, w)]
                    au = aT_sb[mp][b][ds(64 * hp, 64), cs]
                    if h % 2 == 0:
                        nc.scalar.copy(dd, pv_ps[h][64:65, cs])
                        nc.vector.tensor_copy(au, pv_ps[h][0:64, cs])
                    else:
                        nc.vector.tensor_copy(dd, pv_ps[h][64:65, cs])
                        nc.scalar.copy(au, pv_ps[h][0:64, cs])
                for co in (0, 512):
                    nc.vector.reciprocal_approx_fast(
                        rcp[:, ds(co + c0, w)], dnm[:, ds(co + c0, w)]
                    )

            def norm_finish(b, c0=0, c1=512):
                """Stage 2: broadcast reciprocals along partitions via PE
                rank-1 outer products (ones ⊗ r) into PV-pool psum tiles
                (whose previous blocks were evacuated fast), then normalize
                aT in place (SBUF in0, single PSUM in1)."""
                w = c1 - c0
                cs = ds(c0, w)
                rbs = []
                for mp in range(2):
                    rb = pvp.tile([128, 512], F32, tag="pv", name="ps_rb")
                    for hp in range(2):
                        row, co = DROW[2 * mp + hp]
                        nc.tensor.matmul(
                            rb[ds(64 * hp, 64), cs],
                            lhsT=ones_sb[row : row + 1, :],
                            rhs=rcp[row : row + 1, ds(co + c0, w)],
                            start=True,
                            stop=True,
                        )
                    rbs.append(rb)
                for h in range(4):
                    mp, hp = h // 2, h % 2
                    au = aT_sb[mp][b][ds(64 * hp, 64), cs]
                    nc.vector.tensor_mul(au, au, rbs[mp][ds(64 * hp, 64), cs])

            def outproj_half(b, half):
                for t in range(4 * b + 2 * half, 4 * b + 2 * half + 2):
                    ysb = yp.tile([128, C], BF, tag="y", name="ysb")
                    pst = scp.tile([128, 1024], F32, tag="sc", name="ps_y")
                    for n2 in range(2):
                        for kj in range(2):
                            nc.tensor.matmul(
                                pst[:, ts(n2, 512)],
                                lhsT=aT_sb[kj][b][:, ts(t % 4, 128)],
                                rhs=wo_sb[kj][:, ts(n2, 512)],
                                start=(kj == 0),
                                stop=(kj == 1),
                            )
                    nc.scalar.copy(ysb[:, 0:512], pst[:, 0:512])
                    nc.vector.tensor_copy(ysb[:, 512:1024], pst[:, 512:1024])
                    nc.sync.dma_start(y_d[ts(t, 128), :], ysb[:])

            def outproj_block(b):
                for t in range(4 * b, 4 * b + 4):
                    ysb = yp.tile([128, C], BF, tag="y", name="ysb")
                    pst = scp.tile([128, 1024], F32, tag="sc", name="ps_y")
                    for n2 in range(2):
                        for kj in range(2):
                            nc.tensor.matmul(
                                pst[:, ts(n2, 512)],
                                lhsT=aT_sb[kj][b][:, ts(t % 4, 128)],
                                rhs=wo_sb[kj][:, ts(n2, 512)],
                                start=(kj == 0),
                                stop=(kj == 1),
                            )
                    nc.scalar.copy(ysb[:, 0:512], pst[:, 0:512])
                    nc.vector.tensor_copy(ysb[:, 512:1024], pst[:, 512:1024])
                    nc.sync.dma_start(y_d[ts(t, 128), :], ysb[:])

            def attn_step(kt):
                j = kt - 2
                if 0 <= j <= 15:
                    pv_step(j)
                # normalize stage 1 directly after the block's last PV matmul
                # (frees the PV banks); block 3 split at the 384-col boundary
                if 3 <= j <= 15 and j % 4 == 3:
                    if j == 15:
                        norm_start(3, 384, 512)
                    else:
                        norm_start(j // 4)
                elif j == 14:
                    norm_start(3, 0, 384)
                # outproj of the previous block: independent PE work between
                # a block's norm stage 1 and its stage 2
                j2 = kt - 6
                if j2 >= 3 and j2 % 4 == 3 and j2 < 15:
                    outproj_block(j2 // 4)
                if 3 <= j <= 15 and j % 4 == 3 and j < 15:
                    norm_finish(j // 4)
                if kt == 17:
                    norm_finish(3, 0, 384)
                    outproj_half(3, 0)  # tiles 12-13
                if kt == 18:
                    norm_finish(3, 384, 512)
                    outproj_half(3, 1)  # tiles 14-15
                # scores/exp at the END of the step: the pv/outproj matmuls
                # above cover the scores-ring WAR wait on exp(kt-1)
                if kt <= 15:
                    scores_mm(kt)
                    exp_mask(kt)

            # ---- fused emission: projections + attention pipeline ----
            # steps for chunk c become ready once chunk c-1's qT/kT/v exist;
            # they are interleaved between chunk c's projection groups so the
            # PE queue always holds independent work behind each norm chain.
            ready = {0: [], 1: [0, 1], 2: [2, 3, 4, 5], 3: [6, 7, 8, 9]}
            for c in range(NCH):
                groups = []
                for m in range(2):
                    for proj in range(2):
                        groups.append(("qk", m, proj, c))
                for t in range(4 * c, 4 * c + 4):
                    groups.append(("v", t))
                steps = list(ready[c])
                for gi, g in enumerate(groups):
                    if g[0] == "qk":
                        qk_group(*g[1:])
                    else:
                        v_tile(g[1])
                    if gi % 2 == 1 and steps:
                        attn_step(steps.pop(0))
                for kt in steps:
                    attn_step(kt)
            for kt in range(10, 19):
                attn_step(kt)

    nc.compile()
    return nc


def make_masks():
    one = np.ones((128, 128), np.float32)
    maskd = np.triu(one)          # keep iff i >= kk  (diag tile)
    masks_ = np.tril(one, -1)     # keep iff i <  kk  (strict tile)
    md2 = np.concatenate([maskd, maskd], axis=1).astype(BF16)
    ms2 = np.concatenate([masks_, masks_], axis=1).astype(BF16)
    return md2, ms2


def make_in_maps(x, wq, wk, wv, wo):
    x = np.asarray(x, np.float32)
    wq, wk, wv, wo = (np.asarray(a, np.float32) for a in (wq, wk, wv, wo))
    md2, ms2 = make_masks()
    md, ms = md2[:, :128], ms2[:, :128]
    # [maskd|masks|maskd|masks]: matches (head, subtile, col) iteration of the
    # fused mask op; cols 0:256 also serve the per-head diag/strict views
    mask_all = np.hstack([md, ms, md, ms])  # [128, 512]

    def tile_rows(a):  # [1024, W] -> [128, 8*W] (row-blocks side by side)
        return np.hstack([a[i * 128 : (i + 1) * 128] for i in range(a.shape[0] // 128)])

    def chunk_major(a):  # [1024, 2048] -> [128, 4*8*512] chunk-major blocks
        blocks = []
        for n in range(NCH):
            for kt in range(NKT):
                blocks.append(a[kt * 128 : (kt + 1) * 128, n * 512 : (n + 1) * 512])
        return np.hstack(blocks)

    in_maps = []
    for c in range(8):
        b, g = divmod(c, 4)
        sl = slice(g * CL, (g + 1) * CL)
        xTt = chunk_major(np.ascontiguousarray(x[b].T).astype(BF16))
        wt = np.hstack(
            [
                tile_rows(np.ascontiguousarray(w[sl, :].T).astype(BF16))
                for w in (wq, wk, wv)
            ]
        )
        wot = tile_rows(np.ascontiguousarray(wo[:, sl].T).astype(BF16))
        in_maps.append(
            {"xTt": xTt, "wt": wt, "wot": wot, "maskt": mask_all}
        )
    return in_maps


_PROG = None


def _get_prog():
    global _PROG
    if _PROG is None:
        _PROG = build_program()
    return _PROG


def kernel(x, wq, wk, wv, wo, _trace=False, _tmpdir=None):
    from concourse.bass_utils import run_bass_kernel_spmd

    nc = _get_prog()
    in_maps = make_in_maps(x, wq, wk, wv, wo)
    res = run_bass_kernel_spmd(
        nc, in_maps, core_ids=list(range(8)), trace=_trace, tmpdir=_tmpdir
    )
    y = np.zeros((B, T, C), np.float32)
    for c in range(8):
        b = c // 4
        y[b] += res.results[c]["y"].astype(np.float32)
    if _trace:
        kernel._last_results = res
    return y
